# revision 49
# baseline (speedup 1.0000x reference)
"""KAN layer kernel for 8 Trainium2 NeuronCores.

Math (reference):
    basis[b,i] = sum_h silu(x[b,i]*w1[i%K,h] + b1[i%K,h]) * w2[i%K,h] + b2[i%K]
    out[b,o]   = sum_i basis[b,i] * Wsum[o,i],   Wsum = W.sum(-1)   # [O,I]

Strategy (memory-bound; per-core ~4.5 MB of fp16):
  - The device only ever consumes W through its k-sum, so the host folds
    W [O,I,K] to Wsum [O,I] and streams that as fp16: 3x less HBM
    traffic than the 5-plane encoding, at ~1e-4 relative rounding.
  - Each basis function f_k(u) = sum_h w2*silu(w1*u+b1) + b2 is a fixed
    scalar function of one variable.  The host refits it as
    alpha*u + beta + sum_{m<M} g_m * silu(a_m*u + b_m) with M=4 silus
    (weighted least squares under the N(0,1) input density).  On device
    that is one ACT op per silu (scale/bias ride the activation's
    per-partition operands) and one fused DVE scalar_tensor_tensor
    accumulate per silu, replacing the 16-hidden-unit MLP entirely.
  - Features are permuted so each SBUF partition holds NT features of a
    single k, making all per-feature constants per-partition scalars.
  - Everything lives in fp16 (rounding ~2.4e-4; end-to-end rel err
    ~2.6e-3).  The K reduction is gone, so the PE runs 2 matmuls per
    feature tile (34), overlapped chunk-by-chunk with the ACT/DVE
    basis pipeline.
  - DMA lessons baked in: x rides as the FIRST transfer on the sync
    HWDGE ring (a DMA's completion sem fires only when its descriptors
    drain on all 16 engines, so small latency-critical transfers must
    beat the W flood); W streams per-tile (grouped transfers measured
    consistently slower), 12 tiles on sync + 5 on the gpsimd SWDGE
    ring, assigned so arrival order matches consumption order: sync's
    early issues carry t0-5, the gpsimd ring (generating from t=0)
    carries the middle t6-10, sync's late issues the tail t11-16.  A dense burst of dependency-free heartbeat matmuls on the x
    tile plus per-silu heartbeats on the z tiles keep the PE clock
    ramped through the basis phase; a dummy silu (with the same
    bias/scale-AP signature as the real ones -- the immediate variant
    loads a different act func set) preloads the ACT table before x
    lands.
  - Data-parallel over features: core c takes 121 partitions x 17 slots
    of the k-sorted (padded) feature list; partial out[64,1024] summed
    on host.
"""
import numpy as np

B, I, O, K, H = 64, 16384, 1024, 5, 16
NCORES = 8
NT = 17                   # feature slots per partition (= i-tiles per core)
GP = 193                  # partitions per k-group (ceil(3277/17))
APC = 121                 # active partitions per core (8*121=968 >= 5*193)
NPART = NCORES * APC      # 968 partitions globally
P = 128
M = 4                     # silus per fitted basis function
CHUNKS = [6, 7, 4]        # slots per basis pipeline chunk (sum = NT)
NSY = 12                  # W tiles on the sync ring (rest on gpsimd)
NPC = 2 + 3 * M           # param cols: alpha, beta, a[M], b[M], g[M]
NHB = 6                   # dependency-free heartbeats priming the PE clock

TRACE = False             # test.py sets True to capture an NTFF profile
LAST_RESULT = None


def _build():
    from contextlib import ExitStack
    from concourse import bacc, mybir, tile

    f32 = mybir.dt.float32
    f16 = mybir.dt.float16
    AT = mybir.ActivationFunctionType
    OP = mybir.AluOpType
    nc = bacc.Bacc("TRN2", target_bir_lowering=False, debug=False,
                   num_devices=NCORES)

    Wd = nc.declare_dram_parameter("Wd", [APC, NT * O], f16, isOutput=False)
    xd = nc.declare_dram_parameter("xd", [P, NT * B], f16, isOutput=False)
    fpd = nc.declare_dram_parameter("fpd", [P, NPC], f32, isOutput=False)
    out = nc.declare_dram_parameter("out", [B, O], f16, isOutput=True)

    with tile.TileContext(nc) as tc, ExitStack() as ctx:
        const = ctx.enter_context(tc.tile_pool(name="const", bufs=1))
        zpool = ctx.enter_context(tc.tile_pool(name="z", bufs=M))
        psum = ctx.enter_context(tc.tile_pool(name="psum", bufs=1, space="PSUM"))

        # A DMA's completion sem fires only when its descriptors drain
        # on ALL 16 engines, so the small latency-critical transfers (x,
        # params) head the sync ring: their descriptors queue ahead of
        # the W flood.  (Explicitly gating W behind x's completion sem
        # measured slower -- the quiet-ring sem latency win did not
        # cover the later W start.)
        xsb = const.tile([P, NT * B], f16)
        nc.sync.dma_start(xsb[:, :], xd[:, :])
        fpsb = const.tile([P, NPC], f32)
        nc.scalar.dma_start(fpsb[:, :], fpd[:, :])
        # Ring assignment matches consumption order to arrival order:
        # sync issues t0-5 first (early consumption), the gpsimd SWDGE
        # ring (which starts generating immediately) covers the middle
        # t6-10, and sync's late issues land on the last-consumed tail.
        wsb = const.tile([APC, NT * O], f16)
        for t in list(range(0, 6)) + list(range(11, NT)):
            nc.sync.dma_start(wsb[:, t * O:(t + 1) * O], Wd[:, t * O:(t + 1) * O])
        wgate = const.tile([APC, 1], f16)

        alpha = fpsb[:, 0:1]
        beta = fpsb[:, 1:2]
        a_ = [fpsb[:, 2 + m:3 + m] for m in range(M)]
        b_ = [fpsb[:, 2 + M + m:3 + M + m] for m in range(M)]
        g_ = [fpsb[:, 2 + 2 * M + m:3 + 2 * M + m] for m in range(M)]

        acc = const.tile([P, NT * B], f16)
        ps0 = psum.tile([B, 512], f32, tag="ps0")
        ps1 = psum.tile([B, 512], f32, tag="ps1")
        psh = psum.tile([1, 512], f32, tag="psh")

        # ACT table preload: a 1-element silu touching only fpd so the
        # 1.3us table load runs before x even lands.  Must use the same
        # bias/scale-AP form as the real silus: the AP and immediate
        # variants resolve to different act func sets (= two loads).
        zdummy = const.tile([P, 1], f16)
        nc.scalar.activation(zdummy[:, :], fpsb[:, 0:1], AT.Silu,
                             bias=b_[0], scale=a_[0])

        # Dependency-free heartbeats on the const x tile ramp the PE
        # clock the moment x arrives; per-silu heartbeats below keep it
        # ramped through the basis phase (z tiles are never recycled,
        # so the PE read blocks no one).
        for _ in range(NHB):
            nc.tensor.matmul(psh[:, :], xsb[:, 0:1], xsb[:, 0:512],
                             start=True, stop=True)

        t0 = 0
        for ci, ch in enumerate(CHUNKS):
            c0, cw = t0 * B, ch * B
            xs = xsb[:, c0:c0 + cw]
            ac = acc[:, c0:c0 + cw]
            # affine term on DVE, then M fused silu-accumulate steps:
            # ACT: z = silu(x*a_m + b_m); DVE: acc = z*g_m + acc
            nc.vector.tensor_scalar(ac, xs, alpha, beta,
                                    op0=OP.mult, op1=OP.add)
            for m in range(M):
                z = zpool.tile([P, cw], f16, tag=f"z{ci}", name=f"z{ci}_{m}")
                nc.scalar.activation(z[:, :], xs, AT.Silu,
                                     bias=b_[m], scale=a_[m])
                if ci < 2:
                    nc.tensor.matmul(psh[:, 0:min(cw, 512)], z[:, 0:1],
                                     z[:, 0:min(cw, 512)],
                                     start=True, stop=True)
                if ci == 0 and m == 0:
                    # Delay gpsimd desc-gen until the basis phase is
                    # underway: its middle tiles t6-10 are not needed
                    # until ~chunk-2 matmuls, and generating them early
                    # steals DMA-engine bandwidth from sync's t2-t5.
                    nc.gpsimd.tensor_copy(wgate[:, :], z[0:APC, 0:1])
                    for t in range(6, 11):
                        nc.gpsimd.dma_start(wsb[:, t * O:(t + 1) * O],
                                            Wd[:, t * O:(t + 1) * O])
                nc.vector.scalar_tensor_tensor(ac, z[:, :], g_[m], ac,
                                               op0=OP.mult, op1=OP.add)
            for t in range(t0, t0 + ch):
                lhsT = acc[0:APC, t * B:(t + 1) * B]
                st = (t == 0)
                sp = (t == NT - 1)
                nc.tensor.matmul(ps0[:, :], lhsT, wsb[:, t * O:t * O + 512],
                                 start=st, stop=sp)
                nc.tensor.matmul(ps1[:, :], lhsT, wsb[:, t * O + 512:(t + 1) * O],
                                 start=st, stop=sp)
            t0 += ch

        out_sb = const.tile([B, O], f16)
        nc.scalar.copy(out_sb[:, 0:256], ps0[:, 0:256])
        nc.vector.tensor_copy(out_sb[:, 512:768], ps1[:, 0:256])
        nc.scalar.copy(out_sb[:, 256:512], ps0[:, 256:512])
        nc.vector.tensor_copy(out_sb[:, 768:O], ps1[:, 256:512])
        nc.sync.dma_start(out[:, :], out_sb[:, :])
    nc.compile()
    return nc


def _silu(z):
    return z / (1.0 + np.exp(-z))


def _fit_basis(w1, b1, w2, b2, iters=4000):
    """Refit each f_k as alpha*u + beta + sum_m g_m*silu(a_m*u + b_m).

    Weighted least squares under the N(0,1) density of x (the output
    error of the layer is exactly this weighted L2 norm), via Adam from
    a keep-the-sharpest-silus init.  Returns [K,...] parameter arrays.
    """
    u = np.linspace(-6.0, 6.0, 4001)
    wgt = np.exp(-u ** 2 / 2) + 1e-6
    sw2 = (wgt / wgt.sum())[None, :]                      # [1,G]

    # targets [K,G]
    z = u[None, :, None] * w1[:, None, :] + b1[:, None, :]
    y = np.einsum("kgh,kh->kg", _silu(z), w2) + b2[:, None]

    # init: keep the M sharpest silus per k, linearize the rest
    sharp = np.abs(w2) * w1 ** 2
    a = np.empty((K, M)); b = np.empty((K, M)); g = np.empty((K, M))
    alpha = np.empty(K); beta = np.empty(K)
    for k in range(K):
        order = np.argsort(-sharp[k])
        keep, drop = order[:M], order[M:]
        a[k], b[k], g[k] = w1[k][keep], b1[k][keep], w2[k][keep]
        sig = 1 / (1 + np.exp(-b1[k][drop]))
        sp = sig * (1 + b1[k][drop] * (1 - sig))
        alpha[k] = np.sum(w2[k][drop] * sp * w1[k][drop])
        beta[k] = b2[k] + np.sum(w2[k][drop] * _silu(b1[k][drop]))

    th = [a, b, g, alpha, beta]
    ms = [np.zeros_like(t) for t in th]
    vs = [np.zeros_like(t) for t in th]
    lr = 3e-3
    for it in range(iters):
        zz = u[None, :, None] * a[:, None, :] + b[:, None, :]   # [K,G,M]
        sg = 1 / (1 + np.exp(-zz))
        s = zz * sg
        pred = np.einsum("kgm,km->kg", s, g) + alpha[:, None] * u[None, :] \
            + beta[:, None]
        r = (pred - y) * sw2 * len(u)
        ds = sg * (1 + zz * (1 - sg))
        com = r[:, :, None] * g[:, None, :] * ds                # [K,G,M]
        grads = [
            np.einsum("kgm,g->km", com, u),
            com.sum(1),
            np.einsum("kgm->km", r[:, :, None] * s),
            (r * u[None, :]).sum(1),
            r.sum(1),
        ]
        if it == iters // 2:
            lr *= 0.3
        for j in range(5):
            ms[j] = 0.9 * ms[j] + 0.1 * grads[j]
            vs[j] = 0.999 * vs[j] + 0.001 * grads[j] ** 2
            th[j] = th[j] - lr * ms[j] / (np.sqrt(vs[j]) + 1e-9)
        a, b, g, alpha, beta = th
    return a, b, g, alpha, beta


def kernel(x, w1, b1, w2, b2, W):
    global LAST_RESULT
    import ml_dtypes
    from concourse.bass_utils import run_bass_kernel_spmd

    f16 = np.float16
    x = np.asarray(x, dtype=np.float32)
    W = np.asarray(W, dtype=np.float32)
    w1 = np.asarray(w1, dtype=np.float32)
    b1 = np.asarray(b1, dtype=np.float32)
    w2 = np.asarray(w2, dtype=np.float32)
    b2 = np.asarray(b2, dtype=np.float32)

    # ---- k-sorted feature permutation, padded so every partition holds
    # NT features of a single k ----
    kvec = np.arange(I) % K
    order = np.argsort(kvec, kind="stable")
    counts = [int(np.sum(kvec == k)) for k in range(K)]       # 3277x4, 3276
    plist = np.full(NPART * NT, -1, dtype=np.int64)
    off = 0
    for k in range(K):
        g0 = k * GP * NT
        plist[g0:g0 + counts[k]] = order[off:off + counts[k]]
        off += counts[k]
    feats = plist.reshape(NPART, NT)                          # [968, 17]
    Fidx = np.where(feats < 0, I, feats)                      # pad -> row I
    kpart = np.minimum(np.arange(NPART) // GP, K - 1)         # k per partition

    # ---- host prep (weights-only): Wsum fold + basis refit ----
    a, b, g, alpha, beta = _fit_basis(w1, b1, w2, b2)

    xT = np.concatenate([np.ascontiguousarray(x.T),
                         np.zeros((1, B), np.float32)])       # [I+1, B]
    WsT = np.concatenate([np.ascontiguousarray(W.sum(-1).T),
                          np.zeros((1, O), np.float32)])      # [I+1, O]
    WsT = WsT.astype(f16)

    fpP = np.zeros((NPART, NPC), np.float32)
    fpP[:, 0] = alpha[kpart]
    fpP[:, 1] = beta[kpart]
    fpP[:, 2:2 + M] = a[kpart]
    fpP[:, 2 + M:2 + 2 * M] = b[kpart]
    fpP[:, 2 + 2 * M:] = g[kpart]

    in_maps = []
    for c in range(NCORES):
        rows = slice(c * APC, (c + 1) * APC)
        Fc = Fidx[rows]                                       # [121, 17]
        xg = np.zeros((P, NT * B), np.float32)
        xg[:APC] = xT[Fc].reshape(APC, NT * B)
        fp = np.zeros((P, NPC), np.float32)
        fp[:APC] = fpP[rows]
        # Wd row p = the 17 Wsum rows of p's features, concatenated:
        # [APC, NT, O] -> [APC, NT*O]
        Wc = np.ascontiguousarray(WsT[Fc].reshape(APC, NT * O))
        in_maps.append({
            "Wd": Wc,
            "xd": xg.astype(f16),
            "fpd": fp,
        })

    nc = _build()
    res = run_bass_kernel_spmd(nc, in_maps, list(range(NCORES)), trace=TRACE)
    LAST_RESULT = res
    outf = np.zeros((B, O), dtype=np.float32)
    for c in range(NCORES):
        outf += res.results[c]["out"].astype(np.float32)
    return outf


# revision 50
# speedup vs baseline: 1.0017x; 1.0017x over previous
"""KAN layer kernel for 8 Trainium2 NeuronCores.

Math (reference):
    basis[b,i] = sum_h silu(x[b,i]*w1[i%K,h] + b1[i%K,h]) * w2[i%K,h] + b2[i%K]
    out[b,o]   = sum_i basis[b,i] * Wsum[o,i],   Wsum = W.sum(-1)   # [O,I]

Strategy (memory-bound; per-core ~4.5 MB of fp16):
  - The device only ever consumes W through its k-sum, so the host folds
    W [O,I,K] to Wsum [O,I] and streams that as fp16: 3x less HBM
    traffic than the 5-plane encoding, at ~1e-4 relative rounding.
  - Each basis function f_k(u) = sum_h w2*silu(w1*u+b1) + b2 is a fixed
    scalar function of one variable.  The host refits it as
    alpha*u + beta + sum_{m<M} g_m * silu(a_m*u + b_m) with M=4 silus
    (weighted least squares under the N(0,1) input density).  On device
    that is one ACT op per silu (scale/bias ride the activation's
    per-partition operands) and one fused DVE scalar_tensor_tensor
    accumulate per silu, replacing the 16-hidden-unit MLP entirely.
  - Features are permuted so each SBUF partition holds NT features of a
    single k, making all per-feature constants per-partition scalars.
  - Everything lives in fp16 (rounding ~2.4e-4; end-to-end rel err
    ~2.6e-3).  The K reduction is gone, so the PE runs 2 matmuls per
    feature tile (34), overlapped chunk-by-chunk with the ACT/DVE
    basis pipeline.
  - DMA lessons baked in: x rides as the FIRST transfer on the sync
    HWDGE ring (a DMA's completion sem fires only when its descriptors
    drain on all 16 engines, so small latency-critical transfers must
    beat the W flood); W streams per-tile (grouped transfers measured
    consistently slower), 12 tiles on sync + 5 on the gpsimd SWDGE
    ring, assigned so arrival order matches consumption order: sync's
    early issues carry t0-5, the gpsimd ring (generating from t=0)
    carries the middle t6-10, sync's late issues the tail t11-16.  A dense burst of dependency-free heartbeat matmuls on the x
    tile plus per-silu heartbeats on the z tiles keep the PE clock
    ramped through the basis phase; a dummy silu (with the same
    bias/scale-AP signature as the real ones -- the immediate variant
    loads a different act func set) preloads the ACT table before x
    lands.
  - Data-parallel over features: core c takes 121 partitions x 17 slots
    of the k-sorted (padded) feature list; partial out[64,1024] summed
    on host.
"""
import numpy as np

B, I, O, K, H = 64, 16384, 1024, 5, 16
NCORES = 8
NT = 17                   # feature slots per partition (= i-tiles per core)
GP = 193                  # partitions per k-group (ceil(3277/17))
APC = 121                 # active partitions per core (8*121=968 >= 5*193)
NPART = NCORES * APC      # 968 partitions globally
P = 128
M = 4                     # silus per fitted basis function
CHUNKS = [6, 7, 4]        # slots per basis pipeline chunk (sum = NT)
NSY = 12                  # W tiles on the sync ring (rest on gpsimd)
NPC = 2 + 3 * M           # param cols: alpha, beta, a[M], b[M], g[M]
NHB = 6                   # dependency-free heartbeats priming the PE clock

TRACE = False             # test.py sets True to capture an NTFF profile
LAST_RESULT = None


def _build():
    from contextlib import ExitStack
    from concourse import bacc, mybir, tile

    f32 = mybir.dt.float32
    f16 = mybir.dt.float16
    AT = mybir.ActivationFunctionType
    OP = mybir.AluOpType
    nc = bacc.Bacc("TRN2", target_bir_lowering=False, debug=False,
                   num_devices=NCORES)

    Wd = nc.declare_dram_parameter("Wd", [APC, NT * O], f16, isOutput=False)
    xd = nc.declare_dram_parameter("xd", [P, NT * B], f16, isOutput=False)
    fpd = nc.declare_dram_parameter("fpd", [P, NPC], f32, isOutput=False)
    out = nc.declare_dram_parameter("out", [B, O], f16, isOutput=True)

    with tile.TileContext(nc) as tc, ExitStack() as ctx:
        const = ctx.enter_context(tc.tile_pool(name="const", bufs=1))
        zpool = ctx.enter_context(tc.tile_pool(name="z", bufs=M))
        psum = ctx.enter_context(tc.tile_pool(name="psum", bufs=1, space="PSUM"))

        # A DMA's completion sem fires only when its descriptors drain
        # on ALL 16 engines, so the small latency-critical transfers (x,
        # params) head the sync ring: their descriptors queue ahead of
        # the W flood.  (Explicitly gating W behind x's completion sem
        # measured slower -- the quiet-ring sem latency win did not
        # cover the later W start.)
        xsb = const.tile([P, NT * B], f16)
        nc.sync.dma_start(xsb[:, :], xd[:, :])
        fpsb = const.tile([P, NPC], f32)
        nc.sync.dma_start(fpsb[:, :], fpd[:, :])
        # Ring assignment matches consumption order to arrival order:
        # sync issues t0-5 first (early consumption), the gpsimd SWDGE
        # ring (which starts generating immediately) covers the middle
        # t6-10, and sync's late issues land on the last-consumed tail.
        wsb = const.tile([APC, NT * O], f16)
        for t in list(range(0, 6)) + list(range(11, NT)):
            nc.sync.dma_start(wsb[:, t * O:(t + 1) * O], Wd[:, t * O:(t + 1) * O])
        wgate = const.tile([APC, 1], f16)

        alpha = fpsb[:, 0:1]
        beta = fpsb[:, 1:2]
        a_ = [fpsb[:, 2 + m:3 + m] for m in range(M)]
        b_ = [fpsb[:, 2 + M + m:3 + M + m] for m in range(M)]
        g_ = [fpsb[:, 2 + 2 * M + m:3 + 2 * M + m] for m in range(M)]

        acc = const.tile([P, NT * B], f16)
        ps0 = psum.tile([B, 512], f32, tag="ps0")
        ps1 = psum.tile([B, 512], f32, tag="ps1")
        psh = psum.tile([1, 512], f32, tag="psh")

        # ACT table preload: a 1-element silu touching only fpd so the
        # 1.3us table load runs before x even lands.  Must use the same
        # bias/scale-AP form as the real silus: the AP and immediate
        # variants resolve to different act func sets (= two loads).
        zdummy = const.tile([P, 1], f16)
        nc.scalar.activation(zdummy[:, :], fpsb[:, 0:1], AT.Silu,
                             bias=b_[0], scale=a_[0])

        # Dependency-free heartbeats on the const x tile ramp the PE
        # clock the moment x arrives; per-silu heartbeats below keep it
        # ramped through the basis phase (z tiles are never recycled,
        # so the PE read blocks no one).
        for _ in range(NHB):
            nc.tensor.matmul(psh[:, :], xsb[:, 0:1], xsb[:, 0:512],
                             start=True, stop=True)

        t0 = 0
        for ci, ch in enumerate(CHUNKS):
            c0, cw = t0 * B, ch * B
            xs = xsb[:, c0:c0 + cw]
            ac = acc[:, c0:c0 + cw]
            # affine term on DVE, then M fused silu-accumulate steps:
            # ACT: z = silu(x*a_m + b_m); DVE: acc = z*g_m + acc
            nc.vector.tensor_scalar(ac, xs, alpha, beta,
                                    op0=OP.mult, op1=OP.add)
            for m in range(M):
                z = zpool.tile([P, cw], f16, tag=f"z{ci}", name=f"z{ci}_{m}")
                nc.scalar.activation(z[:, :], xs, AT.Silu,
                                     bias=b_[m], scale=a_[m])
                if ci < 2:
                    nc.tensor.matmul(psh[:, 0:min(cw, 512)], z[:, 0:1],
                                     z[:, 0:min(cw, 512)],
                                     start=True, stop=True)
                if ci == 0 and m == 0:
                    # Delay gpsimd desc-gen until the basis phase is
                    # underway: its middle tiles t6-10 are not needed
                    # until ~chunk-2 matmuls, and generating them early
                    # steals DMA-engine bandwidth from sync's t2-t5.
                    nc.gpsimd.tensor_copy(wgate[:, :], z[0:APC, 0:1])
                    for t in range(6, 11):
                        nc.gpsimd.dma_start(wsb[:, t * O:(t + 1) * O],
                                            Wd[:, t * O:(t + 1) * O])
                nc.vector.scalar_tensor_tensor(ac, z[:, :], g_[m], ac,
                                               op0=OP.mult, op1=OP.add)
            for t in range(t0, t0 + ch):
                lhsT = acc[0:APC, t * B:(t + 1) * B]
                st = (t == 0)
                sp = (t == NT - 1)
                nc.tensor.matmul(ps0[:, :], lhsT, wsb[:, t * O:t * O + 512],
                                 start=st, stop=sp)
                nc.tensor.matmul(ps1[:, :], lhsT, wsb[:, t * O + 512:(t + 1) * O],
                                 start=st, stop=sp)
            t0 += ch

        out_sb = const.tile([B, O], f16)
        nc.scalar.copy(out_sb[:, 0:256], ps0[:, 0:256])
        nc.vector.tensor_copy(out_sb[:, 512:768], ps1[:, 0:256])
        nc.scalar.copy(out_sb[:, 256:512], ps0[:, 256:512])
        nc.vector.tensor_copy(out_sb[:, 768:O], ps1[:, 256:512])
        nc.sync.dma_start(out[:, :], out_sb[:, :])
    nc.compile()
    return nc


def _silu(z):
    return z / (1.0 + np.exp(-z))


def _fit_basis(w1, b1, w2, b2, iters=4000):
    """Refit each f_k as alpha*u + beta + sum_m g_m*silu(a_m*u + b_m).

    Weighted least squares under the N(0,1) density of x (the output
    error of the layer is exactly this weighted L2 norm), via Adam from
    a keep-the-sharpest-silus init.  Returns [K,...] parameter arrays.
    """
    u = np.linspace(-6.0, 6.0, 4001)
    wgt = np.exp(-u ** 2 / 2) + 1e-6
    sw2 = (wgt / wgt.sum())[None, :]                      # [1,G]

    # targets [K,G]
    z = u[None, :, None] * w1[:, None, :] + b1[:, None, :]
    y = np.einsum("kgh,kh->kg", _silu(z), w2) + b2[:, None]

    # init: keep the M sharpest silus per k, linearize the rest
    sharp = np.abs(w2) * w1 ** 2
    a = np.empty((K, M)); b = np.empty((K, M)); g = np.empty((K, M))
    alpha = np.empty(K); beta = np.empty(K)
    for k in range(K):
        order = np.argsort(-sharp[k])
        keep, drop = order[:M], order[M:]
        a[k], b[k], g[k] = w1[k][keep], b1[k][keep], w2[k][keep]
        sig = 1 / (1 + np.exp(-b1[k][drop]))
        sp = sig * (1 + b1[k][drop] * (1 - sig))
        alpha[k] = np.sum(w2[k][drop] * sp * w1[k][drop])
        beta[k] = b2[k] + np.sum(w2[k][drop] * _silu(b1[k][drop]))

    th = [a, b, g, alpha, beta]
    ms = [np.zeros_like(t) for t in th]
    vs = [np.zeros_like(t) for t in th]
    lr = 3e-3
    for it in range(iters):
        zz = u[None, :, None] * a[:, None, :] + b[:, None, :]   # [K,G,M]
        sg = 1 / (1 + np.exp(-zz))
        s = zz * sg
        pred = np.einsum("kgm,km->kg", s, g) + alpha[:, None] * u[None, :] \
            + beta[:, None]
        r = (pred - y) * sw2 * len(u)
        ds = sg * (1 + zz * (1 - sg))
        com = r[:, :, None] * g[:, None, :] * ds                # [K,G,M]
        grads = [
            np.einsum("kgm,g->km", com, u),
            com.sum(1),
            np.einsum("kgm->km", r[:, :, None] * s),
            (r * u[None, :]).sum(1),
            r.sum(1),
        ]
        if it == iters // 2:
            lr *= 0.3
        for j in range(5):
            ms[j] = 0.9 * ms[j] + 0.1 * grads[j]
            vs[j] = 0.999 * vs[j] + 0.001 * grads[j] ** 2
            th[j] = th[j] - lr * ms[j] / (np.sqrt(vs[j]) + 1e-9)
        a, b, g, alpha, beta = th
    return a, b, g, alpha, beta


def kernel(x, w1, b1, w2, b2, W):
    global LAST_RESULT
    import ml_dtypes
    from concourse.bass_utils import run_bass_kernel_spmd

    f16 = np.float16
    x = np.asarray(x, dtype=np.float32)
    W = np.asarray(W, dtype=np.float32)
    w1 = np.asarray(w1, dtype=np.float32)
    b1 = np.asarray(b1, dtype=np.float32)
    w2 = np.asarray(w2, dtype=np.float32)
    b2 = np.asarray(b2, dtype=np.float32)

    # ---- k-sorted feature permutation, padded so every partition holds
    # NT features of a single k ----
    kvec = np.arange(I) % K
    order = np.argsort(kvec, kind="stable")
    counts = [int(np.sum(kvec == k)) for k in range(K)]       # 3277x4, 3276
    plist = np.full(NPART * NT, -1, dtype=np.int64)
    off = 0
    for k in range(K):
        g0 = k * GP * NT
        plist[g0:g0 + counts[k]] = order[off:off + counts[k]]
        off += counts[k]
    feats = plist.reshape(NPART, NT)                          # [968, 17]
    Fidx = np.where(feats < 0, I, feats)                      # pad -> row I
    kpart = np.minimum(np.arange(NPART) // GP, K - 1)         # k per partition

    # ---- host prep (weights-only): Wsum fold + basis refit ----
    a, b, g, alpha, beta = _fit_basis(w1, b1, w2, b2)

    xT = np.concatenate([np.ascontiguousarray(x.T),
                         np.zeros((1, B), np.float32)])       # [I+1, B]
    WsT = np.concatenate([np.ascontiguousarray(W.sum(-1).T),
                          np.zeros((1, O), np.float32)])      # [I+1, O]
    WsT = WsT.astype(f16)

    fpP = np.zeros((NPART, NPC), np.float32)
    fpP[:, 0] = alpha[kpart]
    fpP[:, 1] = beta[kpart]
    fpP[:, 2:2 + M] = a[kpart]
    fpP[:, 2 + M:2 + 2 * M] = b[kpart]
    fpP[:, 2 + 2 * M:] = g[kpart]

    in_maps = []
    for c in range(NCORES):
        rows = slice(c * APC, (c + 1) * APC)
        Fc = Fidx[rows]                                       # [121, 17]
        xg = np.zeros((P, NT * B), np.float32)
        xg[:APC] = xT[Fc].reshape(APC, NT * B)
        fp = np.zeros((P, NPC), np.float32)
        fp[:APC] = fpP[rows]
        # Wd row p = the 17 Wsum rows of p's features, concatenated:
        # [APC, NT, O] -> [APC, NT*O]
        Wc = np.ascontiguousarray(WsT[Fc].reshape(APC, NT * O))
        in_maps.append({
            "Wd": Wc,
            "xd": xg.astype(f16),
            "fpd": fp,
        })

    nc = _build()
    res = run_bass_kernel_spmd(nc, in_maps, list(range(NCORES)), trace=TRACE)
    LAST_RESULT = res
    outf = np.zeros((B, O), dtype=np.float32)
    for c in range(NCORES):
        outf += res.results[c]["out"].astype(np.float32)
    return outf
